# revision 24
# baseline (speedup 1.0000x reference)
"""Trainium2 Bass kernel for nn_InputRotationWrapper: y = WHT(x) @ W^T + b.

Algebraic fold: WHT (normalized Walsh-Hadamard along feature dim, H symmetric)
commutes into the weight: y = (x H) W^T = x (W H)^T.  The device runs a pure
GEMM  y = x @ Wr^T + b  with Wr = WHT(W) computed once on the host.

On top of the fold, one level of STRASSEN over 2x2x2 blocking of (o, k, t)
cuts the PE matmul count by 1/8 — the kernel is PE-streaming-bound at fp16
(1 moving column/cycle), so this is a direct 12.5% win that neither fp8
(accuracy: e4m3 x,W measures 3.8e-2 rel err vs the 2e-2 gate) nor uint8
(TRN2 silicon zeroes integer matmul products; probed via a NEFF dtype patch)
can reach.

  C = Wr @ x^T = [[C11 C12],[C21 C22]],  A = Wr halves, B = x^T halves
  M1=(A11+A22)(B11+B22) M2=(A21+A22)B11 M3=A11(B12-B22) M4=A22(B21-B11)
  M5=(A11+A12)B22 M6=(A21-A11)(B11+B12) M7=(A12-A22)(B21+B22)
  C11=M1+M4-M5+M7  C12=M3+M5  C21=M2+M4  C22=M1-M2+M3+M6

Per core (1024 tokens, data-parallel over 8 cores): 16 o-block iterations x
7 products x 16-chunk PSUM accumulation = 1792 matmuls of 512 cols (vs 2048
classical) ~ 387 us PE wall.  W-side combos are host-precomputed and
streamed per (product, o-block); x-side combos are SBUF-resident.

DMA economics (measured): one HWDGE queue sustains ~146 GB/s, the three
(gpsimd/scalar/sync) together ~300 GB/s, and a DMA trigger whose ring-WAR
semaphore isn't met blocks its whole queue.  The startup (x-combos + first
W) is the critical window, so:
  - only the 4 linearly independent x-combos are shipped (B11, B22,
    B12-B22, B21-B11 = 8.4 MB instead of 14.7); the other three are derived
    on the idle vector ALU: xc[M1]=B11+B22, xc[M6]=xc[M1]+xc[M3combo],
    xc[M7]=xc[M1]+xc[M4combo],
  - products run in ORDER=(M2,M5,M3,M4,M1,M6,M7) so shipped combos are
    consumed first and the derived ones are needed only from position 4,
  - W alternates gpsimd/scalar by position; x-combo c-halves ride
    scalar/sync ahead of same-position W; sync also carries bias+outputs,
  - the first G=4 o-blocks run j-major / c-outer so each arriving x chunk
    unlocks 4 matmuls during the DMA ramp (G=4 keeps two positions inside
    the 8-tile W ring; larger G ring-blocks next-position W),
  - ~40 dummy matmuls keep the PE HAM clock from re-throttling during the
    ramp.

Eviction: ScalarE copies each product PSUM->SBUF fp16; the vector ALU
recombines with scalar_tensor_tensor (bias fused via the per-partition
scalar operand) ACCUMULATING IN PLACE into the output tiles, so after the
last product of every o-block only evict -> one stt -> DMA remains.
"""
import sys

for _p in ("/opt/trn_rl_repo", "/root/.axon_site/_ro/trn_rl_repo"):
    if _p not in sys.path:
        sys.path.insert(0, _p)

import numpy as np

D = 4096          # feature dim (= rotation size)
TOKENS = 8192     # 4 * 2048
N_CORES = 8
T_CORE = TOKENS // N_CORES   # 1024 tokens per core
P = 128           # partitions
HALF = D // 2     # 2048: o/k half size
KH = HALF // P    # 16 contraction chunks per half
OBH = HALF // P   # 16 output blocks per half
TH = T_CORE // 2  # 512 tokens per t-half (= one matmul moving dim)
NPROD = 7

# product indices (m/M numbering): 0..6 = M1..M7
ORDER = (1, 4, 2, 3, 0, 5, 6)   # emission order; last = M7 -> 1-stt tail
SHIP = {1: 0, 4: 1, 2: 2, 3: 3}  # shipped x-combos -> slot in xc dram tensor

_compiled = None


def _matmul_hadU_np(x: np.ndarray) -> np.ndarray:
    """Normalized WHT along the last axis — exact port of the reference
    recursive-butterfly (K == 1 branch), in float64."""
    n = x.shape[-1]
    shape = x.shape
    v = x.reshape(-1, n, 1)
    while v.shape[1] > 1:
        b_, m, c = v.shape
        v = v.reshape(b_, m // 2, 2, c)
        a, b = v[:, :, 0, :], v[:, :, 1, :]
        v = np.concatenate([a + b, a - b], axis=-1)
    return v.reshape(shape) / np.sqrt(n)


def _build_nc():
    import concourse.tile as tile
    from concourse import bacc, mybir

    dt = mybir.dt
    alu = mybir.AluOpType
    nc = bacc.Bacc(None, target_bir_lowering=False)

    xc_d = nc.dram_tensor("xc", [4, P, KH, TH], dt.float16,
                          kind="ExternalInput")
    wc_d = nc.dram_tensor("wc", [NPROD, OBH, P, KH, P], dt.float16,
                          kind="ExternalInput")
    b_d = nc.dram_tensor("bias", [P, 2 * OBH], dt.float32,
                         kind="ExternalInput")
    y_d = nc.dram_tensor("yt", [D, T_CORE], dt.float16, kind="ExternalOutput")

    G = 4
    WRING = 8   # W tile ring (4 KB/partition each)
    MRING = 20  # staged-product ring (1 KB/partition each)
    ORING = 24  # output-tile ring (in-place accumulators live pos2..pos6)

    with tile.TileContext(nc) as tc:
        with (
            tc.tile_pool(name="xcp", bufs=1) as xcp,
            tc.tile_pool(name="wp", bufs=WRING) as wp,
            tc.tile_pool(name="mp", bufs=MRING) as mp,
            tc.tile_pool(name="op", bufs=ORING) as op,
            tc.tile_pool(name="bp", bufs=1) as bp,
            tc.tile_pool(name="pp", bufs=8, space="PSUM") as pp,
        ):
            b_sb = bp.tile([P, 2 * OBH], dt.float32)

            xc_sb = [
                xcp.tile([P, KH, TH], dt.float16, name=f"xc_{j}")
                for j in range(NPROD)
            ]

            dum = bp.tile([P, 256], dt.float16, tag="dum", name="dum")
            nc.vector.memset(dum[:], 0.0)

            w_tiles = {}

            def w_alloc(j, obp):
                t = wp.tile([P, KH, P], dt.float16, tag="w",
                            name=f"w_{j}_{obp}")
                w_tiles[(j, obp)] = t
                return t

            def w_load(j, obp, eng):
                t = w_alloc(j, obp)
                eng.dma_start(t[:], wc_d[j, obp, :, :, :])
                return t

            def xc_load(j, c0, n, eng=None):
                (eng or nc.scalar).dma_start(
                    xc_sb[j][:, c0:c0 + n, :], xc_d[SHIP[j], :, c0:c0 + n, :])

            # ---- DMA triggers in arrival-need order ----
            nc.sync.dma_start(b_sb[:], b_d[:])
            # pos0 (M2 <- B11) + its W, finely chunked for the DMA ramp
            xc_load(1, 0, 1)
            xc_load(1, 8, 2, eng=nc.sync)
            for gob in range(G):
                t = w_alloc(1, gob)
                nc.gpsimd.dma_start(t[:, 0:4, :], wc_d[1, gob, :, 0:4, :])
            xc_load(1, 1, 1)
            xc_load(1, 10, 2, eng=nc.sync)
            for gob in range(G):
                nc.gpsimd.dma_start(
                    w_tiles[(1, gob)][:, 4:8, :], wc_d[1, gob, :, 4:8, :])
            xc_load(1, 2, 2)
            xc_load(1, 12, 4, eng=nc.sync)
            for gob in range(G):
                nc.gpsimd.dma_start(
                    w_tiles[(1, gob)][:, 8:16, :], wc_d[1, gob, :, 8:16, :])
            xc_load(1, 4, 4)
            # pos1 (M5 <- B22), W on scalar
            xc_load(4, 0, 8)
            xc_load(4, 8, 8, eng=nc.sync)
            for gob in range(G):
                w_load(4, gob, nc.scalar if gob % 2 else nc.sync)
            # pos2 (M3 <- B12-B22), W on gpsimd
            xc_load(2, 0, 8)
            xc_load(2, 8, 8, eng=nc.sync)
            for gob in range(G):
                w_load(2, gob, nc.gpsimd)
            # pos3 (M4 <- B21-B11), W on scalar
            xc_load(3, 0, 8)
            xc_load(3, 8, 8, eng=nc.sync)
            for gob in range(G):
                w_load(3, gob, nc.scalar if gob % 2 else nc.sync)
            # pos4..6 W (their x-combos are derived on-device)
            for gob in range(G):
                w_load(0, gob, nc.gpsimd)
            for gob in range(G):
                w_load(5, gob, nc.scalar if gob % 2 else nc.sync)
            for gob in range(G):
                w_load(6, gob, nc.gpsimd)

            # derive the dependent x-combos on the vector ALU:
            #   xc[M1] = B11+B22 = xc1+xc4
            #   xc[M6] = B11+B12 = xc[M1]+xc2
            #   xc[M7] = B21+B22 = xc[M1]+xc3
            nc.vector.scalar_tensor_tensor(
                xc_sb[0][:], xc_sb[1][:], 0.0, xc_sb[4][:], alu.add, alu.add)
            nc.vector.scalar_tensor_tensor(
                xc_sb[5][:], xc_sb[0][:], 0.0, xc_sb[2][:], alu.add, alu.add)
            nc.vector.scalar_tensor_tensor(
                xc_sb[6][:], xc_sb[0][:], 0.0, xc_sb[3][:], alu.add, alu.add)

            # ---- PE clock warmup through the DMA ramp ----
            ps_warm = pp.tile([P, TH], dt.float32, tag="ps", name="ps_w")
            for _ in range(72):
                nc.tensor.matmul(
                    ps_warm[:, 0:256], dum[:, 0:128], dum[:, 0:256],
                    start=True, stop=True,
                )

            stage = {}

            def evict(j, obp, ps):
                m = mp.tile([P, TH], dt.float16, tag="m", name=f"m_{j}_{obp}")
                nc.scalar.copy(m[:], ps[:])
                stage[(j, obp)] = m
                return m

            def product(j, obp, ps=None):
                if ps is None:
                    ps = pp.tile([P, TH], dt.float32, tag="ps",
                                 name=f"ps_{j}_{obp}")
                wt = w_tiles.pop((j, obp))
                for c in range(KH):
                    nc.tensor.matmul(
                        ps[:], wt[:, c, :], xc_sb[j][:, c, :],
                        start=(c == 0), stop=(c == KH - 1),
                    )
                evict(j, obp, ps)

            # Incremental in-place recombine on the vector ALU, keyed by the
            # just-finished product.  With ORDER=(1,4,2,3,0,5,6):
            #   pos2 (M3):  o12 = (M3+bt)+M5 ->DMA;  o22 = (M3+bb)-M2
            #   pos3 (M4):  o21 = (M2+bb)+M4 ->DMA;  o11 = (M4+bt)-M5
            #   pos4 (M1):  o11 += M1;  o22 += M1
            #   pos5 (M6):  o22 += M6 ->DMA
            #   pos6 (M7):  o11 += M7 ->DMA
            rec = {}

            def recombine_step(obp, j):
                bt = b_sb[:, obp:obp + 1]
                bb = b_sb[:, OBH + obp:OBH + obp + 1]
                m = lambda k: stage[(k, obp)]
                rt = slice(obp * P, (obp + 1) * P)
                rb = slice((OBH + obp) * P, (OBH + obp + 1) * P)
                r = rec.setdefault(obp, {})
                v = nc.vector

                def tl(nm):
                    return op.tile([P, TH], dt.float16, tag="o",
                                   name=f"{nm}_{obp}")

                if j == 2:
                    o12 = tl("o12")
                    v.scalar_tensor_tensor(
                        o12[:], m(2)[:], bt, m(4)[:], alu.add, alu.add)
                    nc.sync.dma_start(y_d[rt, TH:T_CORE], o12[:])
                    r["o22"] = tl("o22")
                    v.scalar_tensor_tensor(
                        r["o22"][:], m(2)[:], bb, m(1)[:], alu.add,
                        alu.subtract)
                elif j == 3:
                    o21 = tl("o21")
                    v.scalar_tensor_tensor(
                        o21[:], m(1)[:], bb, m(3)[:], alu.add, alu.add)
                    nc.sync.dma_start(y_d[rb, 0:TH], o21[:])
                    r["o11"] = tl("o11")
                    v.scalar_tensor_tensor(
                        r["o11"][:], m(3)[:], bt, m(4)[:], alu.add,
                        alu.subtract)
                elif j == 0:
                    v.scalar_tensor_tensor(
                        r["o11"][:], r["o11"][:], 0.0, m(0)[:], alu.add,
                        alu.add)
                    v.scalar_tensor_tensor(
                        r["o22"][:], r["o22"][:], 0.0, m(0)[:], alu.add,
                        alu.add)
                elif j == 5:
                    v.scalar_tensor_tensor(
                        r["o22"][:], r["o22"][:], 0.0, m(5)[:], alu.add,
                        alu.add)
                    nc.sync.dma_start(y_d[rb, TH:T_CORE], r["o22"][:])
                elif j == 6:
                    v.scalar_tensor_tensor(
                        r["o11"][:], r["o11"][:], 0.0, m(6)[:], alu.add,
                        alu.add)
                    nc.sync.dma_start(y_d[rt, 0:TH], r["o11"][:])
                    for k in range(NPROD):
                        del stage[(k, obp)]
                    del rec[obp]

            # ---- startup group: j-major, c-outer across obp 0..G-1 ----
            first = True
            for j in ORDER:
                ps_j = []
                for gob in range(G):
                    if first and gob == 0:
                        ps_j.append(ps_warm)
                    else:
                        ps_j.append(pp.tile(
                            [P, TH], dt.float32, tag="ps",
                            name=f"ps_{j}_{gob}"))
                first = False
                for c in range(KH):
                    for gob in range(G):
                        nc.tensor.matmul(
                            ps_j[gob][:],
                            w_tiles[(j, gob)][:, c, :], xc_sb[j][:, c, :],
                            start=(c == 0), stop=(c == KH - 1),
                        )
                for gob in range(G):
                    evict(j, gob, ps_j[gob])
                for gob in range(G):
                    recombine_step(gob, j)
            for j, gob in list(w_tiles):
                if gob < G:
                    del w_tiles[(j, gob)]

            # startup W for the first steady block
            for j in ORDER:
                w_load(j, G, nc.scalar if j % 2 else nc.gpsimd)

            # ---- steady state: obp-major ----
            for obp in range(G, OBH):
                for j in ORDER:
                    if obp + 1 < OBH:
                        w_load(j, obp + 1,
                               nc.scalar if (j + obp) % 2 else nc.gpsimd)
                    product(j, obp)
                    recombine_step(obp, j)

    nc.compile()
    return nc


def _get_nc():
    global _compiled
    if _compiled is None:
        _compiled = _build_nc()
    return _compiled


def _prep_inputs(x, W, b):
    x = np.asarray(x, dtype=np.float32)
    W = np.asarray(W, dtype=np.float32)
    b = np.asarray(b, dtype=np.float32)

    Wr = _matmul_hadU_np(W.astype(np.float64))  # [o, k] float64
    A11 = Wr[:HALF, :HALF]
    A12 = Wr[:HALF, HALF:]
    A21 = Wr[HALF:, :HALF]
    A22 = Wr[HALF:, HALF:]
    WCs = (A11 + A22, A21 + A22, A11, A22, A11 + A12, A21 - A11, A12 - A22)
    # pack[j][obp, p, c, jo] = WC_j[obp*128 + jo, c*128 + p]
    wc = np.stack([
        w.reshape(OBH, P, KH, P).transpose(0, 3, 2, 1) for w in WCs
    ]).astype(np.float16)
    wc = np.ascontiguousarray(wc)

    b_pack = np.ascontiguousarray(b.reshape(2 * OBH, P).T)  # [128, 32]

    xt = x.reshape(N_CORES, T_CORE, D).transpose(0, 2, 1)  # [core, k, t] f32
    B11 = xt[:, :HALF, :TH]
    B12 = xt[:, :HALF, TH:]
    B21 = xt[:, HALF:, :TH]
    B22 = xt[:, HALF:, TH:]
    # only the 4 independent combos are shipped (SHIP slots: M2,M5,M3,M4)
    XCs = (B11, B22, B12 - B22, B21 - B11)
    # pack[core, s, p, c, t] = XC_s[core, c*128 + p, t]
    xc = np.stack([
        c.reshape(N_CORES, KH, P, TH).transpose(0, 2, 1, 3) for c in XCs
    ], axis=1).astype(np.float16)
    xc = np.ascontiguousarray(xc)

    in_maps = [
        {"xc": xc[i], "wc": wc, "bias": b_pack} for i in range(N_CORES)
    ]
    return in_maps


def _assemble(results):
    # yt per core: [4096 o, 1024 t] fp16 -> y[t, o] fp32
    parts = [r["yt"].T.astype(np.float32) for r in results]
    y = np.concatenate(parts, axis=0)  # [8192, 4096]
    return y.reshape(4, 2048, D)


def _run(x, W, b, **spmd_kwargs):
    from concourse.bass_utils import run_bass_kernel_spmd

    nc = _get_nc()
    in_maps = _prep_inputs(x, W, b)
    res = run_bass_kernel_spmd(nc, in_maps, list(range(N_CORES)), **spmd_kwargs)
    return _assemble(res.results), res


def kernel(x, W, b):
    out, _ = _run(x, W, b)
    return out


# revision 25
# speedup vs baseline: 1.0134x; 1.0134x over previous
"""Trainium2 Bass kernel for nn_InputRotationWrapper: y = WHT(x) @ W^T + b.

Algebraic fold: WHT (normalized Walsh-Hadamard along feature dim, H symmetric)
commutes into the weight: y = (x H) W^T = x (W H)^T.  The device runs a pure
GEMM  y = x @ Wr^T + b  with Wr = WHT(W) computed once on the host.

On top of the fold, one level of STRASSEN over 2x2x2 blocking of (o, k, t)
cuts the PE matmul count by 1/8 — the kernel is PE-streaming-bound at fp16
(1 moving column/cycle), so this is a direct 12.5% win that neither fp8
(accuracy: e4m3 x,W measures 3.8e-2 rel err vs the 2e-2 gate) nor uint8
(TRN2 silicon zeroes integer matmul products; probed via a NEFF dtype patch)
can reach.

  C = Wr @ x^T = [[C11 C12],[C21 C22]],  A = Wr halves, B = x^T halves
  M1=(A11+A22)(B11+B22) M2=(A21+A22)B11 M3=A11(B12-B22) M4=A22(B21-B11)
  M5=(A11+A12)B22 M6=(A21-A11)(B11+B12) M7=(A12-A22)(B21+B22)
  C11=M1+M4-M5+M7  C12=M3+M5  C21=M2+M4  C22=M1-M2+M3+M6

Per core (1024 tokens, data-parallel over 8 cores): 16 o-block iterations x
7 products x 16-chunk PSUM accumulation = 1792 matmuls of 512 cols (vs 2048
classical) ~ 387 us PE wall.  W-side combos are host-precomputed and
streamed per (product, o-block); x-side combos are SBUF-resident.

DMA economics (measured): one HWDGE queue sustains ~146 GB/s, the three
(gpsimd/scalar/sync) together ~300 GB/s, and a DMA trigger whose ring-WAR
semaphore isn't met blocks its whole queue.  The startup (x-combos + first
W) is the critical window, so:
  - only the 4 linearly independent x-combos are shipped (B11, B22,
    B12-B22, B21-B11 = 8.4 MB instead of 14.7); the other three are derived
    on the idle vector ALU: xc[M1]=B11+B22, xc[M6]=xc[M1]+xc[M3combo],
    xc[M7]=xc[M1]+xc[M4combo],
  - products run in ORDER=(M2,M5,M3,M4,M1,M6,M7) so shipped combos are
    consumed first and the derived ones are needed only from position 4,
  - W alternates gpsimd/scalar by position; x-combo c-halves ride
    scalar/sync ahead of same-position W; sync also carries bias+outputs,
  - the first G=4 o-blocks run j-major / c-outer so each arriving x chunk
    unlocks 4 matmuls during the DMA ramp (G=4 keeps two positions inside
    the 8-tile W ring; larger G ring-blocks next-position W),
  - ~40 dummy matmuls keep the PE HAM clock from re-throttling during the
    ramp.

Eviction: ScalarE copies each product PSUM->SBUF fp16; the vector ALU
recombines with scalar_tensor_tensor (bias fused via the per-partition
scalar operand) ACCUMULATING IN PLACE into the output tiles, so after the
last product of every o-block only evict -> one stt -> DMA remains.
"""
import sys

for _p in ("/opt/trn_rl_repo", "/root/.axon_site/_ro/trn_rl_repo"):
    if _p not in sys.path:
        sys.path.insert(0, _p)

import numpy as np

D = 4096          # feature dim (= rotation size)
TOKENS = 8192     # 4 * 2048
N_CORES = 8
T_CORE = TOKENS // N_CORES   # 1024 tokens per core
P = 128           # partitions
HALF = D // 2     # 2048: o/k half size
KH = HALF // P    # 16 contraction chunks per half
OBH = HALF // P   # 16 output blocks per half
TH = T_CORE // 2  # 512 tokens per t-half (= one matmul moving dim)
NPROD = 7

# product indices (m/M numbering): 0..6 = M1..M7
ORDER = (1, 4, 2, 3, 0, 5, 6)   # emission order; last = M7 -> 1-stt tail
SHIP = {1: 0, 4: 1, 2: 2, 3: 3}  # shipped x-combos -> slot in xc dram tensor

_compiled = None


def _matmul_hadU_np(x: np.ndarray) -> np.ndarray:
    """Normalized WHT along the last axis — exact port of the reference
    recursive-butterfly (K == 1 branch), in float64."""
    n = x.shape[-1]
    shape = x.shape
    v = x.reshape(-1, n, 1)
    while v.shape[1] > 1:
        b_, m, c = v.shape
        v = v.reshape(b_, m // 2, 2, c)
        a, b = v[:, :, 0, :], v[:, :, 1, :]
        v = np.concatenate([a + b, a - b], axis=-1)
    return v.reshape(shape) / np.sqrt(n)


def _build_nc():
    import concourse.tile as tile
    from concourse import bacc, mybir

    dt = mybir.dt
    alu = mybir.AluOpType
    nc = bacc.Bacc(None, target_bir_lowering=False)

    xc_d = nc.dram_tensor("xc", [4, P, KH, TH], dt.float16,
                          kind="ExternalInput")
    wc_d = nc.dram_tensor("wc", [NPROD, OBH, P, KH, P], dt.float16,
                          kind="ExternalInput")
    b_d = nc.dram_tensor("bias", [P, 2 * OBH], dt.float32,
                         kind="ExternalInput")
    y_d = nc.dram_tensor("yt", [D, T_CORE], dt.float16, kind="ExternalOutput")

    G = 4
    WRING = 8   # W tile ring (4 KB/partition each)
    MRING = 20  # staged-product ring (1 KB/partition each)
    ORING = 24  # output-tile ring (in-place accumulators live pos2..pos6)

    with tile.TileContext(nc) as tc:
        with (
            tc.tile_pool(name="xcp", bufs=1) as xcp,
            tc.tile_pool(name="wp", bufs=WRING) as wp,
            tc.tile_pool(name="mp", bufs=MRING) as mp,
            tc.tile_pool(name="op", bufs=ORING) as op,
            tc.tile_pool(name="bp", bufs=1) as bp,
            tc.tile_pool(name="pp", bufs=8, space="PSUM") as pp,
        ):
            b_sb = bp.tile([P, 2 * OBH], dt.float32)

            xc_sb = [
                xcp.tile([P, KH, TH], dt.float16, name=f"xc_{j}")
                for j in range(NPROD)
            ]

            dum = bp.tile([P, 256], dt.float16, tag="dum", name="dum")
            nc.vector.memset(dum[:], 0.0)

            w_tiles = {}

            def w_alloc(j, obp):
                t = wp.tile([P, KH, P], dt.float16, tag="w",
                            name=f"w_{j}_{obp}")
                w_tiles[(j, obp)] = t
                return t

            def w_load(j, obp, eng):
                t = w_alloc(j, obp)
                eng.dma_start(t[:], wc_d[j, obp, :, :, :])
                return t

            def xc_load(j, c0, n, eng=None):
                (eng or nc.scalar).dma_start(
                    xc_sb[j][:, c0:c0 + n, :], xc_d[SHIP[j], :, c0:c0 + n, :])

            # ---- DMA triggers in arrival-need order ----
            nc.sync.dma_start(b_sb[:], b_d[:])
            # pos0 (M2 <- B11) + its W, finely chunked for the DMA ramp
            xc_load(1, 0, 1)
            xc_load(1, 8, 2, eng=nc.sync)
            for gob in range(G):
                t = w_alloc(1, gob)
                nc.gpsimd.dma_start(t[:, 0:4, :], wc_d[1, gob, :, 0:4, :])
            xc_load(1, 1, 1)
            xc_load(1, 10, 2, eng=nc.sync)
            for gob in range(G):
                nc.gpsimd.dma_start(
                    w_tiles[(1, gob)][:, 4:8, :], wc_d[1, gob, :, 4:8, :])
            xc_load(1, 2, 2)
            xc_load(1, 12, 4, eng=nc.sync)
            for gob in range(G):
                nc.gpsimd.dma_start(
                    w_tiles[(1, gob)][:, 8:16, :], wc_d[1, gob, :, 8:16, :])
            xc_load(1, 4, 4)
            # pos1 (M5 <- B22), W on scalar
            xc_load(4, 0, 8)
            xc_load(4, 8, 8, eng=nc.sync)
            for gob in range(G):
                w_load(4, gob, nc.scalar)
            # pos2 (M3 <- B12-B22), W on gpsimd
            xc_load(2, 0, 8)
            xc_load(2, 8, 8, eng=nc.sync)
            for gob in range(G):
                w_load(2, gob, nc.gpsimd)
            # pos3 (M4 <- B21-B11), W on scalar
            xc_load(3, 0, 8)
            xc_load(3, 8, 8, eng=nc.sync)
            for gob in range(G):
                w_load(3, gob, nc.scalar)
            # pos4..6 W (their x-combos are derived on-device)
            for gob in range(G):
                w_load(0, gob, nc.gpsimd)
            for gob in range(G):
                w_load(5, gob, nc.scalar)
            for gob in range(G):
                w_load(6, gob, nc.gpsimd)

            # derive the dependent x-combos on the vector ALU:
            #   xc[M1] = B11+B22 = xc1+xc4
            #   xc[M6] = B11+B12 = xc[M1]+xc2
            #   xc[M7] = B21+B22 = xc[M1]+xc3
            nc.vector.scalar_tensor_tensor(
                xc_sb[0][:], xc_sb[1][:], 0.0, xc_sb[4][:], alu.add, alu.add)
            nc.vector.scalar_tensor_tensor(
                xc_sb[5][:], xc_sb[0][:], 0.0, xc_sb[2][:], alu.add, alu.add)
            nc.vector.scalar_tensor_tensor(
                xc_sb[6][:], xc_sb[0][:], 0.0, xc_sb[3][:], alu.add, alu.add)

            # ---- PE clock warmup through the DMA ramp ----
            ps_warm = pp.tile([P, TH], dt.float32, tag="ps", name="ps_w")
            for _ in range(40):
                nc.tensor.matmul(
                    ps_warm[:, 0:256], dum[:, 0:128], dum[:, 0:256],
                    start=True, stop=True,
                )

            stage = {}

            def evict(j, obp, ps):
                m = mp.tile([P, TH], dt.float16, tag="m", name=f"m_{j}_{obp}")
                nc.scalar.copy(m[:], ps[:])
                stage[(j, obp)] = m
                return m

            def product(j, obp, ps=None):
                if ps is None:
                    ps = pp.tile([P, TH], dt.float32, tag="ps",
                                 name=f"ps_{j}_{obp}")
                wt = w_tiles.pop((j, obp))
                for c in range(KH):
                    nc.tensor.matmul(
                        ps[:], wt[:, c, :], xc_sb[j][:, c, :],
                        start=(c == 0), stop=(c == KH - 1),
                    )
                evict(j, obp, ps)

            # Incremental in-place recombine on the vector ALU, keyed by the
            # just-finished product.  With ORDER=(1,4,2,3,0,5,6):
            #   pos2 (M3):  o12 = (M3+bt)+M5 ->DMA;  o22 = (M3+bb)-M2
            #   pos3 (M4):  o21 = (M2+bb)+M4 ->DMA;  o11 = (M4+bt)-M5
            #   pos4 (M1):  o11 += M1;  o22 += M1
            #   pos5 (M6):  o22 += M6 ->DMA
            #   pos6 (M7):  o11 += M7 ->DMA
            rec = {}

            def recombine_step(obp, j):
                bt = b_sb[:, obp:obp + 1]
                bb = b_sb[:, OBH + obp:OBH + obp + 1]
                m = lambda k: stage[(k, obp)]
                rt = slice(obp * P, (obp + 1) * P)
                rb = slice((OBH + obp) * P, (OBH + obp + 1) * P)
                r = rec.setdefault(obp, {})
                v = nc.vector

                def tl(nm):
                    return op.tile([P, TH], dt.float16, tag="o",
                                   name=f"{nm}_{obp}")

                if j == 2:
                    o12 = tl("o12")
                    v.scalar_tensor_tensor(
                        o12[:], m(2)[:], bt, m(4)[:], alu.add, alu.add)
                    nc.sync.dma_start(y_d[rt, TH:T_CORE], o12[:])
                    r["o22"] = tl("o22")
                    v.scalar_tensor_tensor(
                        r["o22"][:], m(2)[:], bb, m(1)[:], alu.add,
                        alu.subtract)
                elif j == 3:
                    o21 = tl("o21")
                    v.scalar_tensor_tensor(
                        o21[:], m(1)[:], bb, m(3)[:], alu.add, alu.add)
                    nc.sync.dma_start(y_d[rb, 0:TH], o21[:])
                    r["o11"] = tl("o11")
                    v.scalar_tensor_tensor(
                        r["o11"][:], m(3)[:], bt, m(4)[:], alu.add,
                        alu.subtract)
                elif j == 0:
                    v.scalar_tensor_tensor(
                        r["o11"][:], r["o11"][:], 0.0, m(0)[:], alu.add,
                        alu.add)
                    v.scalar_tensor_tensor(
                        r["o22"][:], r["o22"][:], 0.0, m(0)[:], alu.add,
                        alu.add)
                elif j == 5:
                    v.scalar_tensor_tensor(
                        r["o22"][:], r["o22"][:], 0.0, m(5)[:], alu.add,
                        alu.add)
                    nc.sync.dma_start(y_d[rb, TH:T_CORE], r["o22"][:])
                elif j == 6:
                    v.scalar_tensor_tensor(
                        r["o11"][:], r["o11"][:], 0.0, m(6)[:], alu.add,
                        alu.add)
                    nc.sync.dma_start(y_d[rt, 0:TH], r["o11"][:])
                    for k in range(NPROD):
                        del stage[(k, obp)]
                    del rec[obp]

            # ---- startup group: j-major, c-outer across obp 0..G-1 ----
            first = True
            for j in ORDER:
                ps_j = []
                for gob in range(G):
                    if first and gob == 0:
                        ps_j.append(ps_warm)
                    else:
                        ps_j.append(pp.tile(
                            [P, TH], dt.float32, tag="ps",
                            name=f"ps_{j}_{gob}"))
                first = False
                for c in range(KH):
                    for gob in range(G):
                        nc.tensor.matmul(
                            ps_j[gob][:],
                            w_tiles[(j, gob)][:, c, :], xc_sb[j][:, c, :],
                            start=(c == 0), stop=(c == KH - 1),
                        )
                for gob in range(G):
                    evict(j, gob, ps_j[gob])
                for gob in range(G):
                    recombine_step(gob, j)
            for j, gob in list(w_tiles):
                if gob < G:
                    del w_tiles[(j, gob)]

            # startup W for the first steady block
            for j in ORDER:
                w_load(j, G, nc.scalar if j % 2 else nc.gpsimd)

            # ---- steady state: obp-major ----
            for obp in range(G, OBH):
                for j in ORDER:
                    if obp + 1 < OBH:
                        w_load(j, obp + 1,
                               nc.scalar if (j + obp) % 2 else nc.gpsimd)
                    product(j, obp)
                    recombine_step(obp, j)

    nc.compile()
    return nc


def _get_nc():
    global _compiled
    if _compiled is None:
        _compiled = _build_nc()
    return _compiled


def _prep_inputs(x, W, b):
    x = np.asarray(x, dtype=np.float32)
    W = np.asarray(W, dtype=np.float32)
    b = np.asarray(b, dtype=np.float32)

    Wr = _matmul_hadU_np(W.astype(np.float64))  # [o, k] float64
    A11 = Wr[:HALF, :HALF]
    A12 = Wr[:HALF, HALF:]
    A21 = Wr[HALF:, :HALF]
    A22 = Wr[HALF:, HALF:]
    WCs = (A11 + A22, A21 + A22, A11, A22, A11 + A12, A21 - A11, A12 - A22)
    # pack[j][obp, p, c, jo] = WC_j[obp*128 + jo, c*128 + p]
    wc = np.stack([
        w.reshape(OBH, P, KH, P).transpose(0, 3, 2, 1) for w in WCs
    ]).astype(np.float16)
    wc = np.ascontiguousarray(wc)

    b_pack = np.ascontiguousarray(b.reshape(2 * OBH, P).T)  # [128, 32]

    xt = x.reshape(N_CORES, T_CORE, D).transpose(0, 2, 1)  # [core, k, t] f32
    B11 = xt[:, :HALF, :TH]
    B12 = xt[:, :HALF, TH:]
    B21 = xt[:, HALF:, :TH]
    B22 = xt[:, HALF:, TH:]
    # only the 4 independent combos are shipped (SHIP slots: M2,M5,M3,M4)
    XCs = (B11, B22, B12 - B22, B21 - B11)
    # pack[core, s, p, c, t] = XC_s[core, c*128 + p, t]
    xc = np.stack([
        c.reshape(N_CORES, KH, P, TH).transpose(0, 2, 1, 3) for c in XCs
    ], axis=1).astype(np.float16)
    xc = np.ascontiguousarray(xc)

    in_maps = [
        {"xc": xc[i], "wc": wc, "bias": b_pack} for i in range(N_CORES)
    ]
    return in_maps


def _assemble(results):
    # yt per core: [4096 o, 1024 t] fp16 -> y[t, o] fp32
    parts = [r["yt"].T.astype(np.float32) for r in results]
    y = np.concatenate(parts, axis=0)  # [8192, 4096]
    return y.reshape(4, 2048, D)


def _run(x, W, b, **spmd_kwargs):
    from concourse.bass_utils import run_bass_kernel_spmd

    nc = _get_nc()
    in_maps = _prep_inputs(x, W, b)
    res = run_bass_kernel_spmd(nc, in_maps, list(range(N_CORES)), **spmd_kwargs)
    return _assemble(res.results), res


def kernel(x, W, b):
    out, _ = _run(x, W, b)
    return out


# revision 26
# speedup vs baseline: 1.0269x; 1.0133x over previous
"""Trainium2 Bass kernel for nn_InputRotationWrapper: y = WHT(x) @ W^T + b.

Algebraic fold: WHT (normalized Walsh-Hadamard along feature dim, H symmetric)
commutes into the weight: y = (x H) W^T = x (W H)^T.  The device runs a pure
GEMM  y = x @ Wr^T + b  with Wr = WHT(W) computed once on the host.

On top of the fold, one level of STRASSEN over 2x2x2 blocking of (o, k, t)
cuts the PE matmul count by 1/8 — the kernel is PE-streaming-bound at fp16
(1 moving column/cycle), so this is a direct 12.5% win that neither fp8
(accuracy: e4m3 x,W measures 3.8e-2 rel err vs the 2e-2 gate) nor uint8
(TRN2 silicon zeroes integer matmul products; probed via a NEFF dtype patch)
can reach.

  C = Wr @ x^T = [[C11 C12],[C21 C22]],  A = Wr halves, B = x^T halves
  M1=(A11+A22)(B11+B22) M2=(A21+A22)B11 M3=A11(B12-B22) M4=A22(B21-B11)
  M5=(A11+A12)B22 M6=(A21-A11)(B11+B12) M7=(A12-A22)(B21+B22)
  C11=M1+M4-M5+M7  C12=M3+M5  C21=M2+M4  C22=M1-M2+M3+M6

Per core (1024 tokens, data-parallel over 8 cores): 16 o-block iterations x
7 products x 16-chunk PSUM accumulation = 1792 matmuls of 512 cols (vs 2048
classical) ~ 387 us PE wall.  W-side combos are host-precomputed and
streamed per (product, o-block); x-side combos are SBUF-resident.

DMA economics (measured): one HWDGE queue sustains ~146 GB/s, the three
(gpsimd/scalar/sync) together ~300 GB/s, and a DMA trigger whose ring-WAR
semaphore isn't met blocks its whole queue.  The startup (x-combos + first
W) is the critical window, so:
  - only the 4 linearly independent x-combos are shipped (B11, B22,
    B12-B22, B21-B11 = 8.4 MB instead of 14.7); the other three are derived
    on the idle vector ALU: xc[M1]=B11+B22, xc[M6]=xc[M1]+xc[M3combo],
    xc[M7]=xc[M1]+xc[M4combo],
  - products run in ORDER=(M2,M5,M3,M4,M1,M6,M7) so shipped combos are
    consumed first and the derived ones are needed only from position 4,
  - W alternates gpsimd/scalar by position; x-combo c-halves ride
    scalar/sync ahead of same-position W; sync also carries bias+outputs,
  - the first G=4 o-blocks run j-major / c-outer so each arriving x chunk
    unlocks 4 matmuls during the DMA ramp (G=4 keeps two positions inside
    the 8-tile W ring; larger G ring-blocks next-position W),
  - ~40 dummy matmuls keep the PE HAM clock from re-throttling during the
    ramp.

Eviction: ScalarE copies each product PSUM->SBUF fp16; the vector ALU
recombines with scalar_tensor_tensor (bias fused via the per-partition
scalar operand) ACCUMULATING IN PLACE into the output tiles, so after the
last product of every o-block only evict -> one stt -> DMA remains.
"""
import sys

for _p in ("/opt/trn_rl_repo", "/root/.axon_site/_ro/trn_rl_repo"):
    if _p not in sys.path:
        sys.path.insert(0, _p)

import numpy as np

D = 4096          # feature dim (= rotation size)
TOKENS = 8192     # 4 * 2048
N_CORES = 8
T_CORE = TOKENS // N_CORES   # 1024 tokens per core
P = 128           # partitions
HALF = D // 2     # 2048: o/k half size
KH = HALF // P    # 16 contraction chunks per half
OBH = HALF // P   # 16 output blocks per half
TH = T_CORE // 2  # 512 tokens per t-half (= one matmul moving dim)
NPROD = 7

# product indices (m/M numbering): 0..6 = M1..M7
ORDER = (1, 4, 2, 3, 0, 5, 6)   # emission order; last = M7 -> 1-stt tail
SHIP = {1: 0, 4: 1, 2: 2, 3: 3}  # shipped x-combos -> slot in xc dram tensor

_compiled = None


def _matmul_hadU_np(x: np.ndarray) -> np.ndarray:
    """Normalized WHT along the last axis — exact port of the reference
    recursive-butterfly (K == 1 branch), in float64."""
    n = x.shape[-1]
    shape = x.shape
    v = x.reshape(-1, n, 1)
    while v.shape[1] > 1:
        b_, m, c = v.shape
        v = v.reshape(b_, m // 2, 2, c)
        a, b = v[:, :, 0, :], v[:, :, 1, :]
        v = np.concatenate([a + b, a - b], axis=-1)
    return v.reshape(shape) / np.sqrt(n)


def _build_nc():
    import concourse.tile as tile
    from concourse import bacc, mybir

    dt = mybir.dt
    alu = mybir.AluOpType
    nc = bacc.Bacc(None, target_bir_lowering=False)

    xc_d = nc.dram_tensor("xc", [4, P, KH, TH], dt.float16,
                          kind="ExternalInput")
    wc_d = nc.dram_tensor("wc", [NPROD, OBH, P, KH, P], dt.float16,
                          kind="ExternalInput")
    b_d = nc.dram_tensor("bias", [P, 2 * OBH], dt.float32,
                         kind="ExternalInput")
    y_d = nc.dram_tensor("yt", [D, T_CORE], dt.float16, kind="ExternalOutput")

    G = 4
    WRING = 8   # W tile ring (4 KB/partition each)
    MRING = 20  # staged-product ring (1 KB/partition each)
    ORING = 24  # output-tile ring (in-place accumulators live pos2..pos6)

    with tile.TileContext(nc) as tc:
        with (
            tc.tile_pool(name="xcp", bufs=1) as xcp,
            tc.tile_pool(name="wp", bufs=WRING) as wp,
            tc.tile_pool(name="mp", bufs=MRING) as mp,
            tc.tile_pool(name="op", bufs=ORING) as op,
            tc.tile_pool(name="bp", bufs=1) as bp,
            tc.tile_pool(name="pp", bufs=8, space="PSUM") as pp,
        ):
            b_sb = bp.tile([P, 2 * OBH], dt.float32)

            xc_sb = [
                xcp.tile([P, KH, TH], dt.float16, name=f"xc_{j}")
                for j in range(NPROD)
            ]

            dum = bp.tile([P, 256], dt.float16, tag="dum", name="dum")
            nc.vector.memset(dum[:], 0.0)

            w_tiles = {}

            def w_alloc(j, obp):
                t = wp.tile([P, KH, P], dt.float16, tag="w",
                            name=f"w_{j}_{obp}")
                w_tiles[(j, obp)] = t
                return t

            def w_load(j, obp, eng):
                t = w_alloc(j, obp)
                eng.dma_start(t[:], wc_d[j, obp, :, :, :])
                return t

            def xc_load(j, c0, n, eng=None):
                (eng or nc.scalar).dma_start(
                    xc_sb[j][:, c0:c0 + n, :], xc_d[SHIP[j], :, c0:c0 + n, :])

            # ---- DMA triggers in arrival-need order ----
            nc.sync.dma_start(b_sb[:], b_d[:])
            # pos0 (M2 <- B11) + its W, finely chunked for the DMA ramp
            xc_load(1, 0, 1)
            xc_load(1, 8, 2, eng=nc.sync)
            for gob in range(G):
                t = w_alloc(1, gob)
                nc.gpsimd.dma_start(t[:, 0:4, :], wc_d[1, gob, :, 0:4, :])
            xc_load(1, 1, 1)
            xc_load(1, 10, 2, eng=nc.sync)
            for gob in range(G):
                nc.gpsimd.dma_start(
                    w_tiles[(1, gob)][:, 4:8, :], wc_d[1, gob, :, 4:8, :])
            xc_load(1, 2, 2)
            xc_load(1, 12, 4, eng=nc.sync)
            for gob in range(G):
                nc.gpsimd.dma_start(
                    w_tiles[(1, gob)][:, 8:16, :], wc_d[1, gob, :, 8:16, :])
            xc_load(1, 4, 4)
            # pos1 (M5 <- B22), W on scalar
            xc_load(4, 0, 4)
            xc_load(4, 8, 4, eng=nc.sync)
            xc_load(4, 4, 4)
            xc_load(4, 12, 4, eng=nc.sync)
            for gob in range(G):
                w_load(4, gob, nc.scalar)
            # pos2 (M3 <- B12-B22), W on gpsimd
            xc_load(2, 0, 4)
            xc_load(2, 8, 4, eng=nc.sync)
            xc_load(2, 4, 4)
            xc_load(2, 12, 4, eng=nc.sync)
            for gob in range(G):
                w_load(2, gob, nc.gpsimd)
            # pos3 (M4 <- B21-B11), W on scalar
            xc_load(3, 0, 4)
            xc_load(3, 8, 4, eng=nc.sync)
            xc_load(3, 4, 4)
            xc_load(3, 12, 4, eng=nc.sync)
            for gob in range(G):
                w_load(3, gob, nc.scalar)
            # pos4..6 W (their x-combos are derived on-device)
            for gob in range(G):
                w_load(0, gob, nc.gpsimd)
            for gob in range(G):
                w_load(5, gob, nc.scalar)
            for gob in range(G):
                w_load(6, gob, nc.gpsimd)

            # derive the dependent x-combos on the vector ALU:
            #   xc[M1] = B11+B22 = xc1+xc4
            #   xc[M6] = B11+B12 = xc[M1]+xc2
            #   xc[M7] = B21+B22 = xc[M1]+xc3
            nc.vector.scalar_tensor_tensor(
                xc_sb[0][:], xc_sb[1][:], 0.0, xc_sb[4][:], alu.add, alu.add)
            nc.vector.scalar_tensor_tensor(
                xc_sb[5][:], xc_sb[0][:], 0.0, xc_sb[2][:], alu.add, alu.add)
            nc.vector.scalar_tensor_tensor(
                xc_sb[6][:], xc_sb[0][:], 0.0, xc_sb[3][:], alu.add, alu.add)

            # ---- PE clock warmup through the DMA ramp ----
            ps_warm = pp.tile([P, TH], dt.float32, tag="ps", name="ps_w")
            for _ in range(52):
                nc.tensor.matmul(
                    ps_warm[:, 0:256], dum[:, 0:128], dum[:, 0:256],
                    start=True, stop=True,
                )

            stage = {}

            def evict(j, obp, ps):
                m = mp.tile([P, TH], dt.float16, tag="m", name=f"m_{j}_{obp}")
                nc.scalar.copy(m[:], ps[:])
                stage[(j, obp)] = m
                return m

            def product(j, obp, ps=None):
                if ps is None:
                    ps = pp.tile([P, TH], dt.float32, tag="ps",
                                 name=f"ps_{j}_{obp}")
                wt = w_tiles.pop((j, obp))
                for c in range(KH):
                    nc.tensor.matmul(
                        ps[:], wt[:, c, :], xc_sb[j][:, c, :],
                        start=(c == 0), stop=(c == KH - 1),
                    )
                evict(j, obp, ps)

            # Incremental in-place recombine on the vector ALU, keyed by the
            # just-finished product.  With ORDER=(1,4,2,3,0,5,6):
            #   pos2 (M3):  o12 = (M3+bt)+M5 ->DMA;  o22 = (M3+bb)-M2
            #   pos3 (M4):  o21 = (M2+bb)+M4 ->DMA;  o11 = (M4+bt)-M5
            #   pos4 (M1):  o11 += M1;  o22 += M1
            #   pos5 (M6):  o22 += M6 ->DMA
            #   pos6 (M7):  o11 += M7 ->DMA
            rec = {}

            def recombine_step(obp, j):
                bt = b_sb[:, obp:obp + 1]
                bb = b_sb[:, OBH + obp:OBH + obp + 1]
                m = lambda k: stage[(k, obp)]
                rt = slice(obp * P, (obp + 1) * P)
                rb = slice((OBH + obp) * P, (OBH + obp + 1) * P)
                r = rec.setdefault(obp, {})
                v = nc.vector

                def tl(nm):
                    return op.tile([P, TH], dt.float16, tag="o",
                                   name=f"{nm}_{obp}")

                if j == 2:
                    o12 = tl("o12")
                    v.scalar_tensor_tensor(
                        o12[:], m(2)[:], bt, m(4)[:], alu.add, alu.add)
                    nc.sync.dma_start(y_d[rt, TH:T_CORE], o12[:])
                    r["o22"] = tl("o22")
                    v.scalar_tensor_tensor(
                        r["o22"][:], m(2)[:], bb, m(1)[:], alu.add,
                        alu.subtract)
                elif j == 3:
                    o21 = tl("o21")
                    v.scalar_tensor_tensor(
                        o21[:], m(1)[:], bb, m(3)[:], alu.add, alu.add)
                    nc.sync.dma_start(y_d[rb, 0:TH], o21[:])
                    r["o11"] = tl("o11")
                    v.scalar_tensor_tensor(
                        r["o11"][:], m(3)[:], bt, m(4)[:], alu.add,
                        alu.subtract)
                elif j == 0:
                    v.scalar_tensor_tensor(
                        r["o11"][:], r["o11"][:], 0.0, m(0)[:], alu.add,
                        alu.add)
                    v.scalar_tensor_tensor(
                        r["o22"][:], r["o22"][:], 0.0, m(0)[:], alu.add,
                        alu.add)
                elif j == 5:
                    v.scalar_tensor_tensor(
                        r["o22"][:], r["o22"][:], 0.0, m(5)[:], alu.add,
                        alu.add)
                    nc.sync.dma_start(y_d[rb, TH:T_CORE], r["o22"][:])
                elif j == 6:
                    v.scalar_tensor_tensor(
                        r["o11"][:], r["o11"][:], 0.0, m(6)[:], alu.add,
                        alu.add)
                    nc.sync.dma_start(y_d[rt, 0:TH], r["o11"][:])
                    for k in range(NPROD):
                        del stage[(k, obp)]
                    del rec[obp]

            # ---- startup group: j-major, c-outer across obp 0..G-1 ----
            first = True
            for j in ORDER:
                ps_j = []
                for gob in range(G):
                    if first and gob == 0:
                        ps_j.append(ps_warm)
                    else:
                        ps_j.append(pp.tile(
                            [P, TH], dt.float32, tag="ps",
                            name=f"ps_{j}_{gob}"))
                first = False
                for c in range(KH):
                    for gob in range(G):
                        nc.tensor.matmul(
                            ps_j[gob][:],
                            w_tiles[(j, gob)][:, c, :], xc_sb[j][:, c, :],
                            start=(c == 0), stop=(c == KH - 1),
                        )
                for gob in range(G):
                    evict(j, gob, ps_j[gob])
                for gob in range(G):
                    recombine_step(gob, j)
            for j, gob in list(w_tiles):
                if gob < G:
                    del w_tiles[(j, gob)]

            # startup W for the first steady block
            for j in ORDER:
                w_load(j, G, nc.scalar if j % 2 else nc.gpsimd)

            # ---- steady state: obp-major ----
            for obp in range(G, OBH):
                for j in ORDER:
                    if obp + 1 < OBH:
                        w_load(j, obp + 1,
                               nc.scalar if (j + obp) % 2 else nc.gpsimd)
                    product(j, obp)
                    recombine_step(obp, j)

    nc.compile()
    return nc


def _get_nc():
    global _compiled
    if _compiled is None:
        _compiled = _build_nc()
    return _compiled


def _prep_inputs(x, W, b):
    x = np.asarray(x, dtype=np.float32)
    W = np.asarray(W, dtype=np.float32)
    b = np.asarray(b, dtype=np.float32)

    Wr = _matmul_hadU_np(W.astype(np.float64))  # [o, k] float64
    A11 = Wr[:HALF, :HALF]
    A12 = Wr[:HALF, HALF:]
    A21 = Wr[HALF:, :HALF]
    A22 = Wr[HALF:, HALF:]
    WCs = (A11 + A22, A21 + A22, A11, A22, A11 + A12, A21 - A11, A12 - A22)
    # pack[j][obp, p, c, jo] = WC_j[obp*128 + jo, c*128 + p]
    wc = np.stack([
        w.reshape(OBH, P, KH, P).transpose(0, 3, 2, 1) for w in WCs
    ]).astype(np.float16)
    wc = np.ascontiguousarray(wc)

    b_pack = np.ascontiguousarray(b.reshape(2 * OBH, P).T)  # [128, 32]

    xt = x.reshape(N_CORES, T_CORE, D).transpose(0, 2, 1)  # [core, k, t] f32
    B11 = xt[:, :HALF, :TH]
    B12 = xt[:, :HALF, TH:]
    B21 = xt[:, HALF:, :TH]
    B22 = xt[:, HALF:, TH:]
    # only the 4 independent combos are shipped (SHIP slots: M2,M5,M3,M4)
    XCs = (B11, B22, B12 - B22, B21 - B11)
    # pack[core, s, p, c, t] = XC_s[core, c*128 + p, t]
    xc = np.stack([
        c.reshape(N_CORES, KH, P, TH).transpose(0, 2, 1, 3) for c in XCs
    ], axis=1).astype(np.float16)
    xc = np.ascontiguousarray(xc)

    in_maps = [
        {"xc": xc[i], "wc": wc, "bias": b_pack} for i in range(N_CORES)
    ]
    return in_maps


def _assemble(results):
    # yt per core: [4096 o, 1024 t] fp16 -> y[t, o] fp32
    parts = [r["yt"].T.astype(np.float32) for r in results]
    y = np.concatenate(parts, axis=0)  # [8192, 4096]
    return y.reshape(4, 2048, D)


def _run(x, W, b, **spmd_kwargs):
    from concourse.bass_utils import run_bass_kernel_spmd

    nc = _get_nc()
    in_maps = _prep_inputs(x, W, b)
    res = run_bass_kernel_spmd(nc, in_maps, list(range(N_CORES)), **spmd_kwargs)
    return _assemble(res.results), res


def kernel(x, W, b):
    out, _ = _run(x, W, b)
    return out
